# revision 22
# baseline (speedup 1.0000x reference)
"""Multi-head GAT layer (4 heads, mean-aggregated) + residual + GraphNorm + gelu
on 8 Trainium2 NeuronCores (SPMD, one NEFF on all cores).

v4 strategy (vs v3):
  - Nodes are sharded BY GRAPH (batch id == core id), so GraphNorm is fully
    local: no AllReduce, no collective dead-time.
  - Within a core, nodes are sorted by in-degree and packed into 128-slot dst
    tiles; a node's edges live at its OWN SLOT COLUMN in consecutive 128-col
    edge blocks ("slot-aligned").  The aggregation one-hot therefore becomes
    the IDENTITY (16KB constant, no 21MB m1/m2 DMAs) and a_dst distribution is
    a single identity-matmul per tile against a broadcast-replicated rhs.
  - Softmax is PRE-divided: denominators come from a DVE reduce over the ex
    tile, coef = ex * recip.  1/H (head mean) is folded into W host-side, so
    aggregation is 4 accumulating 64-col matmuls per block into a single
    [128, 64] agg psum -- no per-tile head-combine chain.
  - Pad edge columns carry a poison source vector v with As.T v = -100 so
    exp(leaky(alpha_pad)) == 0 and pads drop out of both numerator and denom.
  - The per-edge coefficient multiply (rhs = xl * coef) runs on DVE as one
    tensor_tensor per quad (4 blocks, [128, 1024] f32 psum).
"""
import numpy as np
import ml_dtypes

_BF16 = ml_dtypes.bfloat16
_FP8 = ml_dtypes.float8_e4m3
_F32 = np.float32

N, F, C, H, E, B = 100000, 128, 64, 4, 1200000, 8
NCORE = 8
NEG = 0.2
EPS = 1e-5
POISON = -100.0
GBCAP = 64          # max edge blocks per DMA group
QUAD = 4            # xl blocks per psum quad
OFF_MOD = 7         # quads with (ctr % OFF_MOD) < OFF_CUT take the
OFF_CUT = 2         # ACT-evacuate + gpsimd-multiply path (DVE relief)
PEND = 2            # agg matmuls deferred by this many quads (PE pipelining)


def _cdiv(a, b):
    return (a + b - 1) // b


def _host_prep(x, edge_index, batch, W, att_src, att_dst, bias_gat, res_W,
               res_b, gn_weight, gn_bias, gn_mean_scale):
    x = np.asarray(x, _F32)
    W = np.asarray(W, _F32)
    att_src = np.asarray(att_src, _F32)
    att_dst = np.asarray(att_dst, _F32)
    res_W = np.asarray(res_W, _F32)
    batch = np.asarray(batch).astype(np.int64)

    # fused right matrix [F, 328] = [(W/H).T | As.T | Ad.T | res_W.T]
    W3 = W.reshape(H, C, F)
    As = (att_src[:, :, None] * W3).sum(1)           # [H, F]
    Ad = (att_dst[:, :, None] * W3).sum(1)
    rc = np.concatenate([(W.T / H), As.T, Ad.T, res_W.T], axis=1).astype(_BF16)
    xT_bf = x.T.astype(_BF16)                        # [F, N]

    # poison vector: As.T v = POISON for every head -> exp(leaky(alpha)) == 0
    G4 = As @ As.T if False else (As @ As.T)         # [4,4]? As is [H,F]
    # As rows are per-head; want As @ v = POISON*1 with v in span(As.T)
    v = As.T @ np.linalg.solve(As @ As.T, np.full(H, POISON, _F32))
    v_bf = v.astype(_BF16)                           # [F]

    # ---- edges (+ self loops) ----
    loop = np.arange(N, dtype=np.int64)
    src = np.concatenate([np.asarray(edge_index[0]), loop]).astype(np.int64)
    dst = np.concatenate([np.asarray(edge_index[1]), loop]).astype(np.int64)
    deg = np.bincount(dst, minlength=N)              # >=1 due to self loop

    # ---- per-graph node ordering: degree-desc, 128-slot tiles ----
    n_tile = np.empty(N, np.int64)
    n_slot = np.empty(N, np.int64)
    own_all, rowidx_all, nst_all, ng_all = [], [], [], []
    for g in range(NCORE):
        nodes = np.where(batch == g)[0]
        nodes = nodes[np.argsort(-deg[nodes], kind="stable")]
        r = np.arange(len(nodes))
        n_tile[nodes] = r // 128
        n_slot[nodes] = r % 128
        own_all.append(nodes)
        rowidx_all.append(r)                          # row = tile*128+slot = r
        nst_g = deg[nodes[::128]]                     # max deg per tile
        nst_all.append(nst_g.astype(np.int64))
        ng_all.append(len(nodes))
    NT = max(_cdiv(n, 128) for n in ng_all)
    NB = np.zeros(NT, np.int64)
    for g in range(NCORE):
        NB[:len(nst_all[g])] = np.maximum(NB[:len(nst_all[g])], nst_all[g])
    NB = np.maximum(NB, 1)
    TOTB = int(NB.sum())
    blk0 = np.zeros(NT + 1, np.int64)
    blk0[1:] = np.cumsum(NB)
    NBMAX = int(NB.max())

    # groups of tiles for DMA batching; first group is a single tile so the
    # PE starts working as early as possible
    groups, cur, acc = [[0]], [], 0
    for t in range(1, NT):
        if cur and acc + NB[t] > GBCAP:
            groups.append(cur)
            cur, acc = [], 0
        cur.append(t)
        acc += int(NB[t])
    if cur:
        groups.append(cur)
    MAXGB = max(int(NB[np.array(gt)].sum()) for gt in groups)
    MAXNG = max(len(gt) for gt in groups)

    # ---- per-edge slot-aligned columns ----
    order = np.argsort(dst, kind="stable")
    s_src, s_dst = src[order], dst[order]
    dstart = np.zeros(N + 1, np.int64)
    dstart[1:] = np.cumsum(deg)
    j_within = np.arange(len(s_dst)) - dstart[s_dst]
    col = (blk0[n_tile[s_dst]] + j_within) * 128 + n_slot[s_dst]
    e_core = batch[s_dst]

    in_maps = []
    for g in range(NCORE):
        xedgeT = np.empty((F, TOTB * 128), _BF16)
        xedgeT[:] = v_bf[:, None]
        m = e_core == g
        xedgeT[:, col[m]] = xT_bf[:, s_src[m]]
        xTo = np.zeros((F, NT * 128), _BF16)
        nodes, r = own_all[g], rowidx_all[g]
        xTo[:, r] = xT_bf[:, nodes]
        vmask = np.zeros((128, NT), _BF16)
        vmask[r % 128, r // 128] = 1.0
        # gnp: gw | gb | gms | gms(2-gms) | bias_fold | (1-gms) | 1/n | eps
        gnp = np.zeros((1, 6 * C + 2), _F32)
        gnp[0, 0:C] = np.asarray(gn_weight, _F32)
        gnp[0, C:2 * C] = np.asarray(gn_bias, _F32)
        gms = np.asarray(gn_mean_scale, _F32)
        gnp[0, 2 * C:3 * C] = gms
        gnp[0, 3 * C:4 * C] = gms * (2.0 - gms)
        gnp[0, 4 * C:5 * C] = (np.asarray(bias_gat, _F32)
                               + np.asarray(res_b, _F32))
        gnp[0, 5 * C:6 * C] = 1.0 - gms
        gnp[0, 6 * C] = 1.0 / ng_all[g]
        gnp[0, 6 * C + 1] = EPS
        in_maps.append({"xedgeT": xedgeT, "xTo": xTo, "vmask": vmask,
                        "gnp": gnp})

    ident = np.eye(128, dtype=_FP8)
    alpha_t = np.full((128, 1), NEG, _F32)
    for m in in_maps:
        m.update({"rc": rc, "ident": ident, "alpha_t": alpha_t})

    cfg = {
        "NT": NT, "NB": [int(b) for b in NB], "TOTB": TOTB,
        "blk0": [int(b) for b in blk0], "NBMAX": NBMAX,
        "groups": groups, "MAXGB": MAXGB, "MAXNG": MAXNG,
        "own_all": own_all, "rowidx_all": rowidx_all,
    }
    return cfg, in_maps


def _build_nc(cfg):
    import concourse.bacc as bacc
    import concourse.mybir as mybir
    import concourse.tile as tile

    AF = mybir.ActivationFunctionType
    OP = mybir.AluOpType
    AX = mybir.AxisListType
    f32 = mybir.dt.float32
    bf16 = mybir.dt.bfloat16
    fp8 = mybir.dt.float8e4

    NT = cfg["NT"]
    NB = cfg["NB"]
    blk0 = cfg["blk0"]
    NBMAX = cfg["NBMAX"]
    groups = cfg["groups"]
    MAXGB, MAXNG = cfg["MAXGB"], cfg["MAXNG"]
    A0 = NBMAX * 4                      # offset of phase-1 cols inside lr psum

    nc = bacc.Bacc("TRN2", target_bir_lowering=False)

    xedgeT = nc.declare_dram_parameter("xedgeT", [F, cfg["TOTB"] * 128], bf16, isOutput=False)
    xTo = nc.declare_dram_parameter("xTo", [F, NT * 128], bf16, isOutput=False)
    rc = nc.declare_dram_parameter("rc", [F, 328], bf16, isOutput=False)
    ident = nc.declare_dram_parameter("ident", [128, 128], fp8, isOutput=False)
    vmask = nc.declare_dram_parameter("vmask", [128, NT], bf16, isOutput=False)
    alpha_t = nc.declare_dram_parameter("alpha_t", [128, 1], f32, isOutput=False)
    gnp = nc.declare_dram_parameter("gnp", [1, 6 * C + 2], f32, isOutput=False)
    out = nc.declare_dram_parameter("out", [NT * 128, C], f32, isOutput=True)

    with tile.TileContext(nc) as tc:
        with (
            tc.tile_pool(name="const", bufs=1) as cp,
            tc.tile_pool(name="persist", bufs=1) as pers,
            tc.tile_pool(name="xload", bufs=2) as xp,
            tc.tile_pool(name="xe", bufs=2) as xep,
            tc.tile_pool(name="rhsp", bufs=4) as rhp,
            tc.tile_pool(name="small", bufs=6) as smp,
        ):
            rc_sb = cp.tile([F, 328], bf16)
            nc.sync.dma_start(rc_sb[:], rc[:])
            id_sb = cp.tile([128, 128], fp8)
            nc.sync.dma_start(id_sb[:], ident[:])
            vm_sb = cp.tile([128, NT], bf16)
            nc.sync.dma_start(vm_sb[:], vmask[:])
            al_sb = cp.tile([128, 1], f32)
            nc.sync.dma_start(al_sb[:], alpha_t[:])
            gn_sb = cp.tile([1, 6 * C + 2], f32)
            nc.sync.dma_start(gn_sb[:], gnp[:])

            h_sb = pers.tile([128, NT * C], bf16)
            sq_sb = pers.tile([128, NT * C], bf16)
            resid_sb = pers.tile([128, NT * C], bf16)

            with (
                tc.tile_pool(name="psum_lr", bufs=2, space="PSUM") as plr,
                tc.tile_pool(name="psum_xl", bufs=2, space="PSUM") as pxl,
                tc.tile_pool(name="psum_agg", bufs=2, space="PSUM") as pag,
                tc.tile_pool(name="zpool", bufs=2) as zp,
            ):
                quad_ctr = [0]
                for gtiles in groups:
                    t0, ng = gtiles[0], len(gtiles)
                    gb = sum(NB[t] for t in gtiles)
                    xe = xep.tile([F, MAXGB * 128], bf16, tag="xe")
                    nc.sync.dma_start(xe[:, 0:gb * 128],
                                      xedgeT[:, blk0[t0] * 128:(blk0[t0] + gb) * 128])
                    xo = xp.tile([F, MAXNG * 128], bf16, tag="xo")
                    nc.sync.dma_start(xo[:, 0:ng * 128],
                                      xTo[:, t0 * 128:(t0 + ng) * 128])
                    for ti, t in enumerate(gtiles):
                        nb = NB[t]
                        j0 = blk0[t] - blk0[t0]
                        # ---- phase 1: owned sweep (a_dst + residual) ----
                        lr = plr.tile([128, A0 + 68], f32, tag="lr")
                        nc.tensor.matmul(lr[:, A0:A0 + 68],
                                         lhsT=xo[:, ti * 128:(ti + 1) * 128],
                                         rhs=rc_sb[:, 260:328],
                                         start=True, stop=True,
                                         skip_group_check=True)
                        adstb = smp.tile([128, 4], bf16, tag="adstb")
                        nc.scalar.activation(out=adstb[:], in_=lr[:, A0:A0 + 4],
                                             func=AF.Copy)
                        nc.scalar.activation(
                            out=resid_sb[:, t * C:(t + 1) * C],
                            in_=lr[:, A0 + 4:A0 + 68], func=AF.Copy)
                        adrep = smp.tile([128, NBMAX * 4], bf16, tag="adrep")
                        nc.gpsimd.tensor_copy(
                            adrep[:, 0:nb * 4]
                            .rearrange("p (j h) -> p h j", h=4),
                            adstb[:].to_broadcast([128, 4, nb]))
                        # ---- pass A: alpha = a_src(edges) + a_dst ----
                        # a_src matmuls first (pure PE work, any write sets
                        # has_written); the adst broadcast-matmul accumulates
                        # last, by which time the adrep chain has completed --
                        # the PE never stalls waiting for it.
                        for jb in range(nb):
                            nc.tensor.matmul(
                                lr[:, 4 * jb:4 * jb + 4],
                                lhsT=xe[:, (j0 + jb) * 128:(j0 + jb + 1) * 128],
                                rhs=rc_sb[:, 256:260], start=(jb == 0),
                                stop=False,
                                skip_group_check=True)
                        nc.tensor.matmul(lr[:, 0:4 * nb], lhsT=id_sb[:],
                                         rhs=adrep[:, 0:nb * 4],
                                         start=False, stop=True,
                                         skip_group_check=True)
                        nc.scalar.activation(out=lr[:, 0:4 * nb],
                                             in_=lr[:, 0:4 * nb],
                                             func=AF.Prelu,
                                             alpha=al_sb[:, 0:1])
                        ex2 = smp.tile([128, 4 * NBMAX], bf16, tag="ex2")
                        nc.scalar.activation(
                            out=ex2[:].rearrange("p (h j) -> p j h", j=NBMAX)
                            [:, 0:nb, :],
                            in_=lr[:, 0:4 * nb]
                            .rearrange("p (j h) -> p j h", h=4),
                            func=AF.Exp)
                        denom = smp.tile([128, 4], f32, tag="denom")
                        nc.vector.tensor_reduce(
                            out=denom[:],
                            in_=ex2[:].rearrange("p (h j) -> p h j", h=4)
                            [:, :, 0:nb],
                            axis=AX.X, op=OP.add)
                        rcp = smp.tile([128, 4], f32, tag="rcp")
                        nc.vector.reciprocal(rcp[:], denom[:])
                        coef = smp.tile([128, 4 * NBMAX], bf16, tag="coef")
                        nc.gpsimd.tensor_tensor(
                            out=coef[:].rearrange("p (h j) -> p h j", h=4)
                            [:, :, 0:nb],
                            in0=ex2[:].rearrange("p (h j) -> p h j", h=4)
                            [:, :, 0:nb],
                            in1=rcp[:].to_broadcast([128, 4, nb]),
                            op=OP.mult)
                        # ---- pass B: xl quads, coef multiply, aggregate ----
                        # agg matmuls are deferred PEND quads so the PE (in
                        # order) never stalls behind the multiply engines.
                        agg = pag.tile([128, C], f32, tag="agg")
                        pend = []

                        def emit_agg(ent):
                            eq0, enq, erhs = ent
                            for u in range(enq):
                                for h in range(H):
                                    nc.tensor.matmul(
                                        agg[:],
                                        lhsT=id_sb[:],
                                        rhs=erhs[:, u * 256 + h * C:
                                                 u * 256 + (h + 1) * C],
                                        start=(eq0 + u == 0 and h == 0),
                                        stop=(eq0 + u == nb - 1
                                              and h == H - 1),
                                        skip_group_check=True)

                        for q0 in range(0, nb, QUAD):
                            nq = min(QUAD, nb - q0)
                            psq = pxl.tile([128, QUAD * 256], f32, tag="xlps")
                            for u in range(nq):
                                nc.tensor.matmul(
                                    psq[:, u * 256:(u + 1) * 256],
                                    lhsT=xe[:, (j0 + q0 + u) * 128:
                                            (j0 + q0 + u + 1) * 128],
                                    rhs=rc_sb[:, 0:256],
                                    start=True, stop=True,
                                    skip_group_check=True)
                            rhsq = rhp.tile([128, QUAD * 256], bf16, tag="rhs")
                            coef_ap = (coef[:]
                                       .rearrange("p (h j) -> p j h", j=NBMAX)
                                       [:, q0:q0 + nq, :]
                                       .to_broadcast([128, nq, 4, C]))
                            if quad_ctr[0] % OFF_MOD < OFF_CUT:
                                zq = zp.tile([128, QUAD * 256], bf16, tag="zq")
                                nc.scalar.activation(out=zq[:, 0:nq * 256],
                                                     in_=psq[:, 0:nq * 256],
                                                     func=AF.Copy)
                                nc.gpsimd.tensor_tensor(
                                    out=rhsq[:, 0:nq * 256]
                                    .rearrange("p (j h c) -> p j h c",
                                               h=4, c=C),
                                    in0=zq[:, 0:nq * 256]
                                    .rearrange("p (j h c) -> p j h c",
                                               h=4, c=C),
                                    in1=coef_ap, op=OP.mult)
                            else:
                                nc.vector.tensor_tensor(
                                    out=rhsq[:, 0:nq * 256]
                                    .rearrange("p (j h c) -> p j h c",
                                               h=4, c=C),
                                    in0=psq[:, 0:nq * 256]
                                    .rearrange("p (j h c) -> p j h c",
                                               h=4, c=C),
                                    in1=coef_ap, op=OP.mult)
                            quad_ctr[0] += 1
                            pend.append((q0, nq, rhsq))
                            if len(pend) > PEND:
                                emit_agg(pend.pop(0))
                        for ent in pend:
                            emit_agg(ent)
                        # ---- finish tile ----
                        hsl = h_sb[:, t * C:(t + 1) * C]
                        nc.vector.tensor_tensor(
                            out=hsl, in0=agg[:],
                            in1=resid_sb[:, t * C:(t + 1) * C], op=OP.add)
                        nc.gpsimd.tensor_tensor(
                            out=sq_sb[:, t * C:(t + 1) * C],
                            in0=hsl, in1=hsl, op=OP.mult)

            # ---- phase 2.5: graphnorm stats (local, no collective) ----
            with tc.tile_pool(name="psum_st", bufs=1, space="PSUM") as pst:
                stats = pst.tile([1, 2 * C], f32)
                for t in range(NT):
                    nc.tensor.matmul(stats[:, 0:C], lhsT=vm_sb[:, t:t + 1],
                                     rhs=h_sb[:, t * C:(t + 1) * C],
                                     start=(t == 0), stop=(t == NT - 1),
                                     skip_group_check=True)
                    nc.tensor.matmul(stats[:, C:2 * C], lhsT=vm_sb[:, t:t + 1],
                                     rhs=sq_sb[:, t * C:(t + 1) * C],
                                     start=(t == 0), stop=(t == NT - 1),
                                     skip_group_check=True)
                gw = gn_sb[:, 0:C]
                gb = gn_sb[:, C:2 * C]
                gms = gn_sb[:, 2 * C:3 * C]
                gms2m = gn_sb[:, 3 * C:4 * C]
                biasf = gn_sb[:, 4 * C:5 * C]
                invc = gn_sb[:, 6 * C:6 * C + 1]
                epsc = gn_sb[:, 6 * C + 1:6 * C + 2]
                # stats are of h' = h - bias; fold bias back exactly:
                # mean = m' + bias ; Eh2 = E2' + mean^2 - m'^2
                mp = smp.tile([1, C], f32, tag="mp")
                nc.vector.tensor_scalar(out=mp[:], in0=stats[:, 0:C],
                                        scalar1=invc, scalar2=None, op0=OP.mult)
                eh2 = smp.tile([1, C], f32, tag="eh2")
                nc.vector.tensor_scalar(out=eh2[:], in0=stats[:, C:2 * C],
                                        scalar1=invc, scalar2=None, op0=OP.mult)
                mean = smp.tile([1, C], f32, tag="mean")
                nc.vector.tensor_tensor(out=mean[:], in0=mp[:], in1=biasf,
                                        op=OP.add)
                msq = smp.tile([1, C], f32, tag="msq")
                nc.vector.tensor_tensor(out=msq[:], in0=mean[:], in1=mean[:],
                                        op=OP.mult)
                mpsq = smp.tile([1, C], f32, tag="mpsq")
                nc.vector.tensor_tensor(out=mpsq[:], in0=mp[:], in1=mp[:],
                                        op=OP.mult)
                nc.vector.tensor_tensor(out=eh2[:], in0=eh2[:], in1=msq[:],
                                        op=OP.add)
                nc.vector.tensor_tensor(out=eh2[:], in0=eh2[:], in1=mpsq[:],
                                        op=OP.subtract)
                tm = smp.tile([1, C], f32, tag="tm")
                nc.vector.tensor_tensor(out=tm[:], in0=msq[:], in1=gms2m,
                                        op=OP.mult)
                var = smp.tile([1, C], f32, tag="var")
                nc.vector.tensor_tensor(out=var[:], in0=eh2[:], in1=tm[:],
                                        op=OP.subtract)
                std = smp.tile([1, C], f32, tag="std")
                nc.scalar.activation(out=std[:], in_=var[:], func=AF.Sqrt,
                                     bias=epsc)
                nc.vector.reciprocal(std[:], std[:])
                abf = smp.tile([1, 2 * C], f32, tag="abf")
                nc.vector.tensor_tensor(out=abf[:, 0:C], in0=gw, in1=std[:],
                                        op=OP.mult)
                # b = gb + a*(bias - mean*gms)
                nc.vector.tensor_tensor(out=tm[:], in0=mean[:], in1=gms,
                                        op=OP.mult)
                nc.vector.tensor_tensor(out=tm[:], in0=biasf, in1=tm[:],
                                        op=OP.subtract)
                nc.vector.tensor_tensor(out=tm[:], in0=abf[:, 0:C], in1=tm[:],
                                        op=OP.mult)
                nc.vector.tensor_tensor(out=abf[:, C:2 * C], in0=gb, in1=tm[:],
                                        op=OP.add)
                # replicate a and b 4x on partition 0, then broadcast to all
                # partitions: ab4 = [a a a a | b b b b]
                ab8 = smp.tile([1, 8 * C], bf16, tag="ab8")
                for r in range(4):
                    nc.vector.tensor_copy(ab8[:, r * C:(r + 1) * C],
                                          abf[:, 0:C])
                    nc.vector.tensor_copy(ab8[:, (4 + r) * C:(5 + r) * C],
                                          abf[:, C:2 * C])
                ab4 = pers.tile([128, 8 * C], bf16)
                nc.gpsimd.partition_broadcast(ab4[:], ab8[:])

            # ---- phase 3: normalize + gelu + store (4 tiles per op) ----
            G3 = 4
            with tc.tile_pool(name="obufp", bufs=3) as obp:
                for g0 in range(0, NT, G3):
                    ng = min(G3, NT - g0)
                    obuf = obp.tile([128, G3 * C], f32, tag="ob")
                    t1 = smp.tile([128, G3 * C], bf16, tag="t1")
                    nc.gpsimd.tensor_tensor(
                        out=t1[:, 0:ng * C],
                        in0=h_sb[:, g0 * C:(g0 + ng) * C],
                        in1=ab4[:, 0:ng * C], op=OP.mult)
                    t2 = smp.tile([128, G3 * C], bf16, tag="t2")
                    nc.vector.tensor_tensor(
                        out=t2[:, 0:ng * C], in0=t1[:, 0:ng * C],
                        in1=ab4[:, 4 * C:(4 + ng) * C], op=OP.add)
                    nc.scalar.activation(out=obuf[:, 0:ng * C],
                                         in_=t2[:, 0:ng * C],
                                         func=AF.Gelu_apprx_tanh)
                    nc.sync.dma_start(
                        out[g0 * 128:(g0 + ng) * 128, :]
                        .rearrange("(g p) c -> p g c", p=128),
                        obuf[:, 0:ng * C]
                        .rearrange("p (g c) -> p g c", c=C))

    nc.compile()
    return nc


def _gather(cfg, outs):
    full = np.empty((N, C), _F32)
    for g in range(NCORE):
        full[cfg["own_all"][g]] = outs[g][cfg["rowidx_all"][g]]
    return full


def kernel(**inputs):
    from concourse.bass_utils import run_bass_kernel_spmd

    cfg, in_maps = _host_prep(**inputs)
    nc = _build_nc(cfg)
    res = run_bass_kernel_spmd(nc, in_maps, core_ids=list(range(NCORE)))
    return _gather(cfg, [res.results[k]["out"] for k in range(NCORE)])


# revision 25
# speedup vs baseline: 1.0668x; 1.0668x over previous
"""Multi-head GAT layer (4 heads, mean-aggregated) + residual + GraphNorm + gelu
on 8 Trainium2 NeuronCores (SPMD, one NEFF on all cores).

v4 strategy (vs v3):
  - Nodes are sharded BY GRAPH (batch id == core id), so GraphNorm is fully
    local: no AllReduce, no collective dead-time.
  - Within a core, nodes are sorted by in-degree and packed into 128-slot dst
    tiles; a node's edges live at its OWN SLOT COLUMN in consecutive 128-col
    edge blocks ("slot-aligned").  The aggregation one-hot therefore becomes
    the IDENTITY (16KB constant, no 21MB m1/m2 DMAs) and a_dst distribution is
    a single identity-matmul per tile against a broadcast-replicated rhs.
  - Softmax is PRE-divided: denominators come from a DVE reduce over the ex
    tile, coef = ex * recip.  1/H (head mean) is folded into W host-side, so
    aggregation is 4 accumulating 64-col matmuls per block into a single
    [128, 64] agg psum -- no per-tile head-combine chain.
  - Pad edge columns carry a poison source vector v with As.T v = -100 so
    exp(leaky(alpha_pad)) == 0 and pads drop out of both numerator and denom.
  - The per-edge coefficient multiply (rhs = xl * coef) runs on DVE as one
    tensor_tensor per quad (4 blocks, [128, 1024] f32 psum).
"""
import numpy as np
import ml_dtypes

_BF16 = ml_dtypes.bfloat16
_FP8 = ml_dtypes.float8_e4m3
_F32 = np.float32

N, F, C, H, E, B = 100000, 128, 64, 4, 1200000, 8
NCORE = 8
NEG = 0.2
EPS = 1e-5
POISON = -100.0
GBCAP = 64          # max edge blocks per DMA group
QUAD = 4            # xl blocks per psum quad
PEND = 2            # agg matmuls deferred by this many quads (PE pipelining)


def _cdiv(a, b):
    return (a + b - 1) // b


def _host_prep(x, edge_index, batch, W, att_src, att_dst, bias_gat, res_W,
               res_b, gn_weight, gn_bias, gn_mean_scale):
    x = np.asarray(x, _F32)
    W = np.asarray(W, _F32)
    att_src = np.asarray(att_src, _F32)
    att_dst = np.asarray(att_dst, _F32)
    res_W = np.asarray(res_W, _F32)
    batch = np.asarray(batch).astype(np.int64)

    # fused right matrix [F, 328] = [(W/H).T | As.T | Ad.T | res_W.T]
    W3 = W.reshape(H, C, F)
    As = (att_src[:, :, None] * W3).sum(1)           # [H, F]
    Ad = (att_dst[:, :, None] * W3).sum(1)
    rc = np.concatenate([(W.T / H), As.T, Ad.T, res_W.T], axis=1).astype(_BF16)
    xT_bf = x.T.astype(_BF16)                        # [F, N]

    # poison vector: As @ v = POISON*1 for every head -> exp(leaky(alpha)) == 0
    v = As.T @ np.linalg.solve(As @ As.T, np.full(H, POISON, _F32))
    v_bf = v.astype(_BF16)                           # [F]

    # ---- edges (+ self loops) ----
    loop = np.arange(N, dtype=np.int64)
    src = np.concatenate([np.asarray(edge_index[0]), loop]).astype(np.int64)
    dst = np.concatenate([np.asarray(edge_index[1]), loop]).astype(np.int64)
    deg = np.bincount(dst, minlength=N)              # >=1 due to self loop

    # ---- per-graph node ordering: degree-desc, 128-slot tiles ----
    n_tile = np.empty(N, np.int64)
    n_slot = np.empty(N, np.int64)
    own_all, rowidx_all, nst_all, ng_all = [], [], [], []
    for g in range(NCORE):
        nodes = np.where(batch == g)[0]
        nodes = nodes[np.argsort(-deg[nodes], kind="stable")]
        r = np.arange(len(nodes))
        n_tile[nodes] = r // 128
        n_slot[nodes] = r % 128
        own_all.append(nodes)
        rowidx_all.append(r)                          # row = tile*128+slot = r
        nst_g = deg[nodes[::128]]                     # max deg per tile
        nst_all.append(nst_g.astype(np.int64))
        ng_all.append(len(nodes))
    NT = max(_cdiv(n, 128) for n in ng_all)
    NB = np.zeros(NT, np.int64)
    for g in range(NCORE):
        NB[:len(nst_all[g])] = np.maximum(NB[:len(nst_all[g])], nst_all[g])
    NB = np.maximum(NB, 1)
    TOTB = int(NB.sum())
    blk0 = np.zeros(NT + 1, np.int64)
    blk0[1:] = np.cumsum(NB)
    NBMAX = int(NB.max())

    # groups of tiles for DMA batching; first group is a single tile so the
    # PE starts working as early as possible
    groups, cur, acc = [[0]], [], 0
    for t in range(1, NT):
        if cur and acc + NB[t] > GBCAP:
            groups.append(cur)
            cur, acc = [], 0
        cur.append(t)
        acc += int(NB[t])
    if cur:
        groups.append(cur)
    MAXGB = max(int(NB[np.array(gt)].sum()) for gt in groups)
    MAXNG = max(len(gt) for gt in groups)

    # ---- per-edge slot-aligned columns ----
    order = np.argsort(dst, kind="stable")
    s_src, s_dst = src[order], dst[order]
    dstart = np.zeros(N + 1, np.int64)
    dstart[1:] = np.cumsum(deg)
    j_within = np.arange(len(s_dst)) - dstart[s_dst]
    col = (blk0[n_tile[s_dst]] + j_within) * 128 + n_slot[s_dst]
    e_core = batch[s_dst]

    in_maps = []
    for g in range(NCORE):
        xedgeT = np.empty((F, TOTB * 128), _BF16)
        xedgeT[:] = v_bf[:, None]
        m = e_core == g
        xedgeT[:, col[m]] = xT_bf[:, s_src[m]]
        xTo = np.zeros((F, NT * 128), _BF16)
        nodes, r = own_all[g], rowidx_all[g]
        xTo[:, r] = xT_bf[:, nodes]
        vmask = np.zeros((128, NT), _BF16)
        vmask[r % 128, r // 128] = 1.0
        # gnp: gw | gb | gms | gms(2-gms) | bias_fold | (1-gms) | 1/n | eps
        gnp = np.zeros((1, 6 * C + 2), _F32)
        gnp[0, 0:C] = np.asarray(gn_weight, _F32)
        gnp[0, C:2 * C] = np.asarray(gn_bias, _F32)
        gms = np.asarray(gn_mean_scale, _F32)
        gnp[0, 2 * C:3 * C] = gms
        gnp[0, 3 * C:4 * C] = gms * (2.0 - gms)
        gnp[0, 4 * C:5 * C] = (np.asarray(bias_gat, _F32)
                               + np.asarray(res_b, _F32))
        gnp[0, 5 * C:6 * C] = 1.0 - gms
        gnp[0, 6 * C] = 1.0 / ng_all[g]
        gnp[0, 6 * C + 1] = EPS
        in_maps.append({"xedgeT": xedgeT, "xTo": xTo, "vmask": vmask,
                        "gnp": gnp})

    ident = np.eye(128, dtype=_FP8)
    alpha_t = np.full((128, 1), NEG, _F32)
    for m in in_maps:
        m.update({"rc": rc, "ident": ident, "alpha_t": alpha_t})

    cfg = {
        "NT": NT, "NB": [int(b) for b in NB], "TOTB": TOTB,
        "blk0": [int(b) for b in blk0], "NBMAX": NBMAX,
        "groups": groups, "MAXGB": MAXGB, "MAXNG": MAXNG,
        "own_all": own_all, "rowidx_all": rowidx_all,
    }
    return cfg, in_maps


def _build_nc(cfg):
    import concourse.bacc as bacc
    import concourse.mybir as mybir
    import concourse.tile as tile

    AF = mybir.ActivationFunctionType
    OP = mybir.AluOpType
    AX = mybir.AxisListType
    f32 = mybir.dt.float32
    bf16 = mybir.dt.bfloat16
    fp8 = mybir.dt.float8e4

    NT = cfg["NT"]
    NB = cfg["NB"]
    blk0 = cfg["blk0"]
    NBMAX = cfg["NBMAX"]
    groups = cfg["groups"]
    MAXGB, MAXNG = cfg["MAXGB"], cfg["MAXNG"]
    A0 = NBMAX * 4                      # offset of phase-1 cols inside lr psum

    nc = bacc.Bacc("TRN2", target_bir_lowering=False)

    xedgeT = nc.declare_dram_parameter("xedgeT", [F, cfg["TOTB"] * 128], bf16, isOutput=False)
    xTo = nc.declare_dram_parameter("xTo", [F, NT * 128], bf16, isOutput=False)
    rc = nc.declare_dram_parameter("rc", [F, 328], bf16, isOutput=False)
    ident = nc.declare_dram_parameter("ident", [128, 128], fp8, isOutput=False)
    vmask = nc.declare_dram_parameter("vmask", [128, NT], bf16, isOutput=False)
    alpha_t = nc.declare_dram_parameter("alpha_t", [128, 1], f32, isOutput=False)
    gnp = nc.declare_dram_parameter("gnp", [1, 6 * C + 2], f32, isOutput=False)
    out = nc.declare_dram_parameter("out", [NT * 128, C], f32, isOutput=True)

    with tile.TileContext(nc) as tc:
        with (
            tc.tile_pool(name="const", bufs=1) as cp,
            tc.tile_pool(name="persist", bufs=1) as pers,
            tc.tile_pool(name="xload", bufs=2) as xp,
            tc.tile_pool(name="xe", bufs=2) as xep,
            tc.tile_pool(name="rhsp", bufs=4) as rhp,
            tc.tile_pool(name="small", bufs=6) as smp,
        ):
            rc_sb = cp.tile([F, 328], bf16)
            nc.sync.dma_start(rc_sb[:], rc[:])
            id_sb = cp.tile([128, 128], fp8)
            nc.sync.dma_start(id_sb[:], ident[:])
            vm_sb = cp.tile([128, NT], bf16)
            nc.sync.dma_start(vm_sb[:], vmask[:])
            al_sb = cp.tile([128, 1], f32)
            nc.sync.dma_start(al_sb[:], alpha_t[:])
            gn_sb = cp.tile([1, 6 * C + 2], f32)
            nc.sync.dma_start(gn_sb[:], gnp[:])

            h_sb = pers.tile([128, NT * C], bf16)
            sq_sb = pers.tile([128, NT * C], bf16)
            resid_sb = pers.tile([128, NT * C], bf16)

            with (
                tc.tile_pool(name="psum_lr", bufs=2, space="PSUM") as plr,
                tc.tile_pool(name="psum_xl", bufs=2, space="PSUM") as pxl,
                tc.tile_pool(name="psum_agg", bufs=2, space="PSUM") as pag,
                tc.tile_pool(name="zpool", bufs=2) as zp,
            ):
                for gtiles in groups:
                    t0, ng = gtiles[0], len(gtiles)
                    gb = sum(NB[t] for t in gtiles)
                    xe = xep.tile([F, MAXGB * 128], bf16, tag="xe")
                    nc.sync.dma_start(xe[:, 0:gb * 128],
                                      xedgeT[:, blk0[t0] * 128:(blk0[t0] + gb) * 128])
                    xo = xp.tile([F, MAXNG * 128], bf16, tag="xo")
                    nc.sync.dma_start(xo[:, 0:ng * 128],
                                      xTo[:, t0 * 128:(t0 + ng) * 128])
                    for ti, t in enumerate(gtiles):
                        nb = NB[t]
                        j0 = blk0[t] - blk0[t0]
                        # ---- phase 1: owned sweep (a_dst + residual) ----
                        lr = plr.tile([128, A0 + 68], f32, tag="lr")
                        nc.tensor.matmul(lr[:, A0:A0 + 68],
                                         lhsT=xo[:, ti * 128:(ti + 1) * 128],
                                         rhs=rc_sb[:, 260:328],
                                         start=True, stop=True,
                                         skip_group_check=True)
                        adstb = smp.tile([128, 4], bf16, tag="adstb")
                        nc.scalar.activation(out=adstb[:], in_=lr[:, A0:A0 + 4],
                                             func=AF.Copy)
                        nc.scalar.activation(
                            out=resid_sb[:, t * C:(t + 1) * C],
                            in_=lr[:, A0 + 4:A0 + 68], func=AF.Copy)
                        adrep = smp.tile([128, NBMAX * 4], bf16, tag="adrep")
                        nc.gpsimd.tensor_copy(
                            adrep[:, 0:nb * 4]
                            .rearrange("p (j h) -> p h j", h=4),
                            adstb[:].to_broadcast([128, 4, nb]))
                        # ---- pass A: alpha = a_src(edges) + a_dst ----
                        # a_src matmuls first (pure PE work, any write sets
                        # has_written); the adst broadcast-matmul accumulates
                        # last, by which time the adrep chain has completed --
                        # the PE never stalls waiting for it.
                        for jb in range(nb):
                            nc.tensor.matmul(
                                lr[:, 4 * jb:4 * jb + 4],
                                lhsT=xe[:, (j0 + jb) * 128:(j0 + jb + 1) * 128],
                                rhs=rc_sb[:, 256:260], start=(jb == 0),
                                stop=False,
                                skip_group_check=True)
                        nc.tensor.matmul(lr[:, 0:4 * nb], lhsT=id_sb[:],
                                         rhs=adrep[:, 0:nb * 4],
                                         start=False, stop=True,
                                         skip_group_check=True)
                        nc.scalar.activation(out=lr[:, 0:4 * nb],
                                             in_=lr[:, 0:4 * nb],
                                             func=AF.Prelu,
                                             alpha=al_sb[:, 0:1])
                        ex2 = smp.tile([128, 4 * NBMAX], bf16, tag="ex2")
                        nc.scalar.activation(
                            out=ex2[:].rearrange("p (h j) -> p j h", j=NBMAX)
                            [:, 0:nb, :],
                            in_=lr[:, 0:4 * nb]
                            .rearrange("p (j h) -> p j h", h=4),
                            func=AF.Exp)
                        denom = smp.tile([128, 4], f32, tag="denom")
                        nc.vector.tensor_reduce(
                            out=denom[:],
                            in_=ex2[:].rearrange("p (h j) -> p h j", h=4)
                            [:, :, 0:nb],
                            axis=AX.X, op=OP.add)
                        rcp = smp.tile([128, 4], f32, tag="rcp")
                        nc.vector.reciprocal(rcp[:], denom[:])
                        coef = smp.tile([128, 4 * NBMAX], bf16, tag="coef")
                        nc.gpsimd.tensor_tensor(
                            out=coef[:].rearrange("p (h j) -> p h j", h=4)
                            [:, :, 0:nb],
                            in0=ex2[:].rearrange("p (h j) -> p h j", h=4)
                            [:, :, 0:nb],
                            in1=rcp[:].to_broadcast([128, 4, nb]),
                            op=OP.mult)
                        # ---- pass B: xl quads, coef multiply, aggregate ----
                        # agg matmuls are deferred PEND quads so the PE (in
                        # order) never stalls behind the multiply engines.
                        agg = pag.tile([128, C], f32, tag="agg")
                        pend = []

                        def emit_agg(ent):
                            eq0, enq, erhs = ent
                            for u in range(enq):
                                for h in range(H):
                                    nc.tensor.matmul(
                                        agg[:],
                                        lhsT=id_sb[:],
                                        rhs=erhs[:, u * 256 + h * C:
                                                 u * 256 + (h + 1) * C],
                                        start=(eq0 + u == 0 and h == 0),
                                        stop=(eq0 + u == nb - 1
                                              and h == H - 1),
                                        skip_group_check=True)

                        for q0 in range(0, nb, QUAD):
                            nq = min(QUAD, nb - q0)
                            psq = pxl.tile([128, QUAD * 256], f32, tag="xlps")
                            for u in range(nq):
                                nc.tensor.matmul(
                                    psq[:, u * 256:(u + 1) * 256],
                                    lhsT=xe[:, (j0 + q0 + u) * 128:
                                            (j0 + q0 + u + 1) * 128],
                                    rhs=rc_sb[:, 0:256],
                                    start=True, stop=True,
                                    skip_group_check=True)
                            rhsq = rhp.tile([128, QUAD * 256], bf16, tag="rhs")
                            coef_v = (coef[:]
                                      .rearrange("p (h j) -> p j h", j=NBMAX))
                            # split full quads 3+1: DVE multiplies blocks 0-2
                            # from psum; ACT evacuates block 3 and gpsimd
                            # multiplies it -- all three finish within the
                            # PE's per-quad matmul time.
                            nd = 3 if nq == QUAD else nq
                            nc.vector.tensor_tensor(
                                out=rhsq[:, 0:nd * 256]
                                .rearrange("p (j h c) -> p j h c", h=4, c=C),
                                in0=psq[:, 0:nd * 256]
                                .rearrange("p (j h c) -> p j h c", h=4, c=C),
                                in1=coef_v[:, q0:q0 + nd, :]
                                .to_broadcast([128, nd, 4, C]),
                                op=OP.mult)
                            if nd < nq:
                                zq = zp.tile([128, 256], bf16, tag="zq")
                                nc.scalar.activation(
                                    out=zq[:],
                                    in_=psq[:, nd * 256:nq * 256],
                                    func=AF.Copy)
                                nc.gpsimd.tensor_tensor(
                                    out=rhsq[:, nd * 256:nq * 256]
                                    .rearrange("p (j h c) -> p j h c",
                                               h=4, c=C),
                                    in0=zq[:]
                                    .rearrange("p (j h c) -> p j h c",
                                               h=4, c=C),
                                    in1=coef_v[:, q0 + nd:q0 + nq, :]
                                    .to_broadcast([128, 1, 4, C]),
                                    op=OP.mult)
                            pend.append((q0, nq, rhsq))
                            if len(pend) > PEND:
                                emit_agg(pend.pop(0))
                        for ent in pend:
                            emit_agg(ent)
                        # ---- finish tile ----
                        hsl = h_sb[:, t * C:(t + 1) * C]
                        nc.vector.tensor_tensor(
                            out=hsl, in0=agg[:],
                            in1=resid_sb[:, t * C:(t + 1) * C], op=OP.add)
                        nc.gpsimd.tensor_tensor(
                            out=sq_sb[:, t * C:(t + 1) * C],
                            in0=hsl, in1=hsl, op=OP.mult)

            # ---- phase 2.5: graphnorm stats (local, no collective) ----
            with tc.tile_pool(name="psum_st", bufs=1, space="PSUM") as pst:
                stats = pst.tile([1, 2 * C], f32)
                for t in range(NT):
                    nc.tensor.matmul(stats[:, 0:C], lhsT=vm_sb[:, t:t + 1],
                                     rhs=h_sb[:, t * C:(t + 1) * C],
                                     start=(t == 0), stop=(t == NT - 1),
                                     skip_group_check=True)
                    nc.tensor.matmul(stats[:, C:2 * C], lhsT=vm_sb[:, t:t + 1],
                                     rhs=sq_sb[:, t * C:(t + 1) * C],
                                     start=(t == 0), stop=(t == NT - 1),
                                     skip_group_check=True)
                gw = gn_sb[:, 0:C]
                gb = gn_sb[:, C:2 * C]
                gms = gn_sb[:, 2 * C:3 * C]
                gms2m = gn_sb[:, 3 * C:4 * C]
                biasf = gn_sb[:, 4 * C:5 * C]
                invc = gn_sb[:, 6 * C:6 * C + 1]
                epsc = gn_sb[:, 6 * C + 1:6 * C + 2]
                # stats are of h' = h - bias; fold bias back exactly:
                # mean = m' + bias ; Eh2 = E2' + mean^2 - m'^2
                mp = smp.tile([1, C], f32, tag="mp")
                nc.vector.tensor_scalar(out=mp[:], in0=stats[:, 0:C],
                                        scalar1=invc, scalar2=None, op0=OP.mult)
                eh2 = smp.tile([1, C], f32, tag="eh2")
                nc.vector.tensor_scalar(out=eh2[:], in0=stats[:, C:2 * C],
                                        scalar1=invc, scalar2=None, op0=OP.mult)
                mean = smp.tile([1, C], f32, tag="mean")
                nc.vector.tensor_tensor(out=mean[:], in0=mp[:], in1=biasf,
                                        op=OP.add)
                msq = smp.tile([1, C], f32, tag="msq")
                nc.vector.tensor_tensor(out=msq[:], in0=mean[:], in1=mean[:],
                                        op=OP.mult)
                mpsq = smp.tile([1, C], f32, tag="mpsq")
                nc.vector.tensor_tensor(out=mpsq[:], in0=mp[:], in1=mp[:],
                                        op=OP.mult)
                nc.vector.tensor_tensor(out=eh2[:], in0=eh2[:], in1=msq[:],
                                        op=OP.add)
                nc.vector.tensor_tensor(out=eh2[:], in0=eh2[:], in1=mpsq[:],
                                        op=OP.subtract)
                tm = smp.tile([1, C], f32, tag="tm")
                nc.vector.tensor_tensor(out=tm[:], in0=msq[:], in1=gms2m,
                                        op=OP.mult)
                var = smp.tile([1, C], f32, tag="var")
                nc.vector.tensor_tensor(out=var[:], in0=eh2[:], in1=tm[:],
                                        op=OP.subtract)
                std = smp.tile([1, C], f32, tag="std")
                nc.scalar.activation(out=std[:], in_=var[:], func=AF.Sqrt,
                                     bias=epsc)
                nc.vector.reciprocal(std[:], std[:])
                abf = smp.tile([1, 2 * C], f32, tag="abf")
                nc.vector.tensor_tensor(out=abf[:, 0:C], in0=gw, in1=std[:],
                                        op=OP.mult)
                # b = gb + a*(bias - mean*gms)
                nc.vector.tensor_tensor(out=tm[:], in0=mean[:], in1=gms,
                                        op=OP.mult)
                nc.vector.tensor_tensor(out=tm[:], in0=biasf, in1=tm[:],
                                        op=OP.subtract)
                nc.vector.tensor_tensor(out=tm[:], in0=abf[:, 0:C], in1=tm[:],
                                        op=OP.mult)
                nc.vector.tensor_tensor(out=abf[:, C:2 * C], in0=gb, in1=tm[:],
                                        op=OP.add)
                # replicate a and b 4x on partition 0, then broadcast to all
                # partitions: ab4 = [a a a a | b b b b]
                ab8 = smp.tile([1, 8 * C], bf16, tag="ab8")
                for r in range(4):
                    nc.vector.tensor_copy(ab8[:, r * C:(r + 1) * C],
                                          abf[:, 0:C])
                    nc.vector.tensor_copy(ab8[:, (4 + r) * C:(5 + r) * C],
                                          abf[:, C:2 * C])
                ab4 = pers.tile([128, 8 * C], bf16)
                nc.gpsimd.partition_broadcast(ab4[:], ab8[:])

            # ---- phase 3: normalize + gelu + store (4 tiles per op) ----
            G3 = 4
            with tc.tile_pool(name="obufp", bufs=3) as obp:
                for g0 in range(0, NT, G3):
                    ng = min(G3, NT - g0)
                    obuf = obp.tile([128, G3 * C], f32, tag="ob")
                    t1 = smp.tile([128, G3 * C], bf16, tag="t1")
                    nc.gpsimd.tensor_tensor(
                        out=t1[:, 0:ng * C],
                        in0=h_sb[:, g0 * C:(g0 + ng) * C],
                        in1=ab4[:, 0:ng * C], op=OP.mult)
                    t2 = smp.tile([128, G3 * C], bf16, tag="t2")
                    nc.vector.tensor_tensor(
                        out=t2[:, 0:ng * C], in0=t1[:, 0:ng * C],
                        in1=ab4[:, 4 * C:(4 + ng) * C], op=OP.add)
                    nc.scalar.activation(out=obuf[:, 0:ng * C],
                                         in_=t2[:, 0:ng * C],
                                         func=AF.Gelu_apprx_tanh)
                    nc.sync.dma_start(
                        out[g0 * 128:(g0 + ng) * 128, :]
                        .rearrange("(g p) c -> p g c", p=128),
                        obuf[:, 0:ng * C]
                        .rearrange("p (g c) -> p g c", c=C))

    nc.compile()
    return nc


def _gather(cfg, outs):
    full = np.empty((N, C), _F32)
    for g in range(NCORE):
        full[cfg["own_all"][g]] = outs[g][cfg["rowidx_all"][g]]
    return full


def kernel(**inputs):
    from concourse.bass_utils import run_bass_kernel_spmd

    cfg, in_maps = _host_prep(**inputs)
    nc = _build_nc(cfg)
    res = run_bass_kernel_spmd(nc, in_maps, core_ids=list(range(NCORE)))
    return _gather(cfg, [res.results[k]["out"] for k in range(NCORE)])
